# revision 18
# baseline (speedup 1.0000x reference)
"""Trainium2 Bass kernel for nn_KANCouplingNet (3-layer KAN MLP, 12-64-64-24).

Each KAN layer: y = silu(h) @ sb + B(s) @ W with s = h/0.4 + 5.5 and B the
cubic B-spline basis (8 functions per channel).  The exact cardinal basis
needs >=2 DVE ops per basis value (the M4 two-cube formula exceeds the 8-op
DVE pipeline), so instead the basis is replaced by 8 quartic bump features
per channel:

    psi_k(s) = T^2 (T + a_k)(T + b),   T = relu(h_k - |s - c_k|)

computed by ONE custom DVE instruction (8 ALU ops, per-partition c/h/a via
scalar slots, b via imm2).  The transform features->basis is re-fit on the
host per channel with empirical s-density weighting (mini-forward on a pixel
subsample); end-to-end validation vs the exact reference gives ~4.5e-4
relative error (gate 2e-2), robust to bf16 features/weights (5.3e-4).

This halves DVE work vs the exact two-cube kernel (1 op/basis value) and
nearly halves the matmul count.  Sharding: pure data parallel over batch
(32 images -> 4 per core); features built per half-batch (2048 px) blocks,
software-pipelined one stage ahead so TensorE never stalls on the DVE.
"""
import numpy as np
import ml_dtypes

import concourse.bacc as bacc
import concourse.bass as bass
import concourse.mybir as mybir
import concourse.tile as tile
from concourse.bass_utils import run_bass_kernel_spmd

FP = mybir.dt.float32
BF = mybir.dt.bfloat16
AFT = mybir.ActivationFunctionType

N_CORES = 8
B_PER_CORE = 4
HW = 64 * 64            # 4096 px per image
NT = 512                # matmul moving dim (one PSUM bank)
NHB = 2048              # feature-block pixels (half batch)
S_SCALE = 2.5
S_BIAS = 5.5
K = 8                   # feature rows per input channel

# Generator params (offline fit; end-to-end 4.5e-4 vs exact reference).
A_ROWS = np.array([-0.3615, -0.4926, -0.5196, -0.5232,
                   -0.5232, -0.5196, -0.4926, -0.3615], np.float64)
B_GLOB = -2.5643
C_ROWS = np.arange(8, dtype=np.float64) + 2.0
H_ROWS = np.full(8, 2.0, np.float64)

# psi is scale-invariant: T(s) = S_SCALE * T'(h) with T' built from raw h via
# c' = (c - S_BIAS)/S_SCALE etc., and psi = S_SCALE^4 * psi'.  The device op
# therefore consumes raw x / raw pre-activations (no s staging act), with the
# S_SCALE^4 factor folded into the matmul weights.
C_DEV = (C_ROWS - S_BIAS) / S_SCALE
H_DEV = H_ROWS / S_SCALE
A_DEV = A_ROWS / S_SCALE
B_DEV = B_GLOB / S_SCALE
W_FOLD = S_SCALE ** 4

_CACHE = {}
_QOP = None


def _register_quartic_op():
    """out = T^2 (T + in1)(T + imm2), T = relu(s1 - |in0 - s0|)."""
    global _QOP
    if _QOP is not None:
        return _QOP
    from concourse import dve_ops
    from concourse.dve_spec import (AluOp, Bin, C0, C1, C2, C3, Spec, Src0,
                                    _spill_c3_to_src1, lower, relu, sq)
    from concourse.dve_uop import DveOpSpec

    for op in dve_ops.OPS:
        if op.name == "KANQ_ANT":
            _QOP = op
            return op

    w = Bin(AluOp.ABSOLUTE_DIFF, Src0, C0)
    T = relu(Bin(AluOp.SUBTRACT, C1, w))
    body = _spill_c3_to_src1(sq(T) * (T + C3) * (T + C2))

    def _ref(in0, in1, s0, s1, imm2):
        in0 = np.asarray(in0, np.float32)
        tail = [1] * (in0.ndim - 1)
        c = np.asarray(s0, np.float32).reshape(-1, *tail)
        h = np.asarray(s1, np.float32).reshape(-1, *tail)
        a = np.asarray(in1, np.float32).reshape(-1, *tail)
        T = np.maximum(h - np.abs(in0 - c), 0.0)
        return (T * T * (T + a) * (T + imm2)).astype(np.float32)

    spec = Spec(body=body, reference=_ref)
    row = dve_ops._CUSTOM_DVE_ROW_BASE + len(dve_ops.OPS)
    shas = {}
    for ver in ("v3", "v4"):
        tmp = DveOpSpec(name="KANQ_ANT", opcode=row,
                        uops=lower(spec, ver=ver), rd1_en=True)
        shas[ver] = tmp.sha(ver)
    op = dve_ops.DveOp("KANQ_ANT", spec, subdim=False, uops_sha=shas)
    dve_ops.OPS.append(op)
    dve_ops._SUB_OPCODE_FOR_NAME[op.name] = row
    dve_ops.CUSTOM_DVE_SPECS[op.name] = spec
    _QOP = op
    return op


# --------------------------- host-side math --------------------------------

def _m4(v):
    u = np.abs(v - 2.0)
    r2 = np.maximum(2.0 - u, 0.0)
    r1 = np.maximum(1.0 - u, 0.0)
    return (r2**3 - 4.0 * r1**3) / 6.0


def _silu(x):
    return x / (1.0 + np.exp(-x))


def _psi(s):
    """s: (N,) -> (N, 8) quartic bump features."""
    T = np.maximum(H_ROWS - np.abs(s[:, None] - C_ROWS), 0.0)
    return T * T * (T + A_ROWS) * (T + B_GLOB)


def _fit_layer(coef, ss, s_samp):
    """Per-channel density-weighted lstsq: features -> spline weights.

    coef: (din, dout, 8); ss: (din, dout); s_samp: (n, din)
    returns Wfeat (din, 8, dout) float64
    """
    din, dout, _ = coef.shape
    sgrid = np.linspace(-3.0, 14.0, 1201)
    Psi = _psi(sgrid)                                   # (S, 8)
    Mtgt = np.stack([_m4(sgrid - g) for g in range(8)], 1)
    Wout = np.zeros((din, K, dout))
    for i in range(din):
        hist, edges = np.histogram(s_samp[:, i], bins=120,
                                   range=(-3.0, 14.0), density=True)
        centers = 0.5 * (edges[:-1] + edges[1:])
        wt = np.interp(sgrid, centers, hist) + 1e-3
        sw = np.sqrt(wt)[:, None]
        C, *_ = np.linalg.lstsq(sw * Psi, sw * Mtgt, rcond=None)  # (8, 8)
        Wout[i] = C @ (coef[i] * ss[i][:, None]).T                # (8, dout)
    return Wout


def _host_weights(inputs):
    """Mini-forward for s-samples + per-layer fits; assemble device arrays."""
    x = np.asarray(inputs["x"], np.float64)
    hs = np.transpose(x, (0, 2, 3, 1)).reshape(-1, 12)
    rng = np.random.default_rng(0)
    samp = hs[rng.choice(hs.shape[0], 16384, replace=False)]

    Ws = []
    h = samp
    for li in range(3):
        coef = np.asarray(inputs[f"coef{li}"], np.float64)
        sb = np.asarray(inputs[f"sb{li}"], np.float64)
        ss = np.asarray(inputs[f"ss{li}"], np.float64)
        s = S_SCALE * h + S_BIAS
        Ws.append(_fit_layer(coef, ss, s))
        # exact forward for next layer's sample distribution
        Bsp = np.stack([_m4(s - g) for g in range(8)], -1)       # (n, din, 8)
        h = _silu(h) @ sb + np.einsum('nig,iog->no', Bsp, coef * ss[:, :, None])

    bf = ml_dtypes.bfloat16
    sb0 = np.asarray(inputs["sb0"], np.float64)
    sb1 = np.asarray(inputs["sb1"], np.float64)
    sb2 = np.asarray(inputs["sb2"], np.float64)

    # L0 stationary [108, 128]: rows 0..95 = (k = p//12, i = p%12) features,
    # rows 96..107 = silu base; cols duplicated (o, o+64).
    w0 = np.zeros((108, 128))
    for p in range(96):
        k, i = p // 12, p % 12
        w0[p, 0:64] = W_FOLD * Ws[0][i, k]
        w0[p, 64:128] = W_FOLD * Ws[0][i, k]
    w0[96:108, 0:64] = sb0
    w0[96:108, 64:128] = sb0

    # mid stationary per page j: [128, mcols]; row p: ch=p%64, k=j+4*(p//64)
    def midw(W, dout, dup):
        mc = 128 if dup else dout
        out = np.zeros((4, 128, mc))
        for j in range(4):
            for grp in range(2):
                k = j + 4 * grp
                blk = W_FOLD * W[:, k, :]              # (64, dout)
                out[j, 64*grp:64*grp+64, 0:dout] = blk
                if dup:
                    out[j, 64*grp:64*grp+64, 64:128] = blk
        return out

    w1 = midw(Ws[1], 64, True)
    w2 = midw(Ws[2], 24, False)
    b1 = np.zeros((64, 128)); b1[:, 0:64] = sb1; b1[:, 64:128] = sb1
    b2 = sb2

    # DVE per-partition scalars (raw-input scale)
    c0v = np.zeros((96, 1), np.float32); h0v = np.zeros((96, 1), np.float32)
    a0v = np.zeros((96, 1), np.float32)
    for p in range(96):
        k = p // 12
        c0v[p], h0v[p], a0v[p] = C_DEV[k], H_DEV[k], A_DEV[k]
    cv = np.zeros((4, 128, 1), np.float32); hv = np.zeros((4, 128, 1), np.float32)
    av = np.zeros((4, 128, 1), np.float32)
    for j in range(4):
        for p in range(128):
            k = j + 4 * (p // 64)
            cv[j, p], hv[j, p], av[j, p] = C_DEV[k], H_DEV[k], A_DEV[k]

    return {
        "w0": w0.astype(bf), "w1": w1.astype(bf), "b1": b1.astype(bf),
        "w2": w2.astype(bf), "b2": b2.astype(bf),
        "c0v": c0v, "h0v": h0v, "a0v": a0v,
        "cv": cv, "hv": hv, "av": av,
    }


# --------------------------- device program --------------------------------

def _build():
    qop = _register_quartic_op()
    nc = bacc.Bacc("TRN2", target_bir_lowering=False, debug=False,
                   enable_asserts=False, num_devices=N_CORES)

    x_d = nc.dram_tensor("x_in", [B_PER_CORE, 12, HW], FP, kind="ExternalInput").ap()
    out_d = nc.dram_tensor("y_out", [B_PER_CORE, 24, HW], FP, kind="ExternalOutput").ap()
    w0_d = nc.dram_tensor("w0", [108, 128], BF, kind="ExternalInput").ap()
    w1_d = nc.dram_tensor("w1", [4, 128, 128], BF, kind="ExternalInput").ap()
    b1_d = nc.dram_tensor("b1", [64, 128], BF, kind="ExternalInput").ap()
    w2_d = nc.dram_tensor("w2", [4, 128, 24], BF, kind="ExternalInput").ap()
    b2_d = nc.dram_tensor("b2", [64, 24], BF, kind="ExternalInput").ap()
    c0_d = nc.dram_tensor("c0v", [96, 1], FP, kind="ExternalInput").ap()
    h0_d = nc.dram_tensor("h0v", [96, 1], FP, kind="ExternalInput").ap()
    a0_d = nc.dram_tensor("a0v", [96, 1], FP, kind="ExternalInput").ap()
    cv_d = nc.dram_tensor("cv", [4, 128, 1], FP, kind="ExternalInput").ap()
    hv_d = nc.dram_tensor("hv", [4, 128, 1], FP, kind="ExternalInput").ap()
    av_d = nc.dram_tensor("av", [4, 128, 1], FP, kind="ExternalInput").ap()

    with tile.TileContext(nc) as tc:
        with (
            tc.tile_pool(name="consts", bufs=1) as cp,
            tc.tile_pool(name="xr", bufs=2) as xp,
            tc.tile_pool(name="f0", bufs=2) as f0p,
            tc.tile_pool(name="hb", bufs=2) as hp,
            tc.tile_pool(name="ps", bufs=2, space="PSUM") as pp,
        ):
            w0 = cp.tile([108, 128], BF, tag="w0")
            nc.sync.dma_start(w0[:], w0_d[:])
            w1 = [cp.tile([128, 128], BF, tag=f"w1_{j}", name=f"w1_{j}") for j in range(4)]
            w2 = [cp.tile([128, 24], BF, tag=f"w2_{j}", name=f"w2_{j}") for j in range(4)]
            for j in range(4):
                nc.sync.dma_start(w1[j][:], w1_d[j])
                nc.sync.dma_start(w2[j][:], w2_d[j])
            b1 = cp.tile([64, 128], BF, tag="b1")
            nc.sync.dma_start(b1[:], b1_d[:])
            b2 = cp.tile([64, 24], BF, tag="b2")
            nc.sync.dma_start(b2[:], b2_d[:])
            c0v = cp.tile([96, 1], FP, tag="c0v"); nc.sync.dma_start(c0v[:], c0_d[:])
            h0v = cp.tile([96, 1], FP, tag="h0v"); nc.sync.dma_start(h0v[:], h0_d[:])
            a0v = cp.tile([96, 1], FP, tag="a0v"); nc.sync.dma_start(a0v[:], a0_d[:])
            cv = [cp.tile([128, 1], FP, tag=f"cv{j}", name=f"cv{j}") for j in range(4)]
            hv = [cp.tile([128, 1], FP, tag=f"hv{j}", name=f"hv{j}") for j in range(4)]
            av = [cp.tile([128, 1], FP, tag=f"av{j}", name=f"av{j}") for j in range(4)]
            for j in range(4):
                nc.sync.dma_start(cv[j][:], cv_d[j])
                nc.sync.dma_start(hv[j][:], hv_d[j])
                nc.sync.dma_start(av[j][:], av_d[j])
            def x_load(b):
                """Issue the x replication DMAs (no compute deps)."""
                xr = xp.tile([96, HW], FP, tag="xr")
                for r in range(8):
                    nc.sync.dma_start(xr[12*r:12*(r+1), :], x_d[b])
                return xr

            def f0_build(xr):
                """f0 features + silu base from replicated raw x."""
                f0t = f0p.tile([108, HW], BF, tag="f0")
                for hh in range(2):
                    hcols = bass.ts(hh, HW // 2)
                    nc.vector._custom_dve(qop, out=f0t[0:96, hcols],
                                          in0=xr[:, hcols], in1=a0v[:],
                                          s0=c0v[:], s1=h0v[:], imm2=B_DEV)
                nc.scalar.activation(f0t[96:108, :], xr[0:12, :], AFT.Silu)
                return f0t

            def stage_A(f0t, hb):
                """L0 matmuls; evacuate h1 (gpsimd) + silu (scalar)."""
                s1 = hp.tile([128, NHB], FP, tag="s1")
                sil1 = hp.tile([64, NHB], BF, tag="sil1")
                for t in range(4):
                    bcols = bass.ts(4*hb + t, NT)
                    lcols = bass.ts(t, NT)
                    ps1 = pp.tile([128, NT], FP, tag="ps1")
                    nc.tensor.matmul(ps1[:], w0[:], f0t[:, bcols],
                                     start=True, stop=True)
                    nc.scalar.activation(s1[:, lcols], ps1[:], AFT.Identity)
                    nc.scalar.activation(sil1[:, lcols], ps1[0:64, :], AFT.Silu)
                return s1, sil1

            def feats(s_t, tag):
                f = [hp.tile([128, NHB], BF, tag=f"{tag}_{j}", name=f"{tag}_{j}")
                     for j in range(4)]
                for j in range(4):
                    nc.vector._custom_dve(qop, out=f[j][:], in0=s_t[:],
                                          in1=av[j][:], s0=cv[j][:],
                                          s1=hv[j][:], imm2=B_DEV)
                return f

            def stage_C(f1, sil1):
                """L1 matmuls; evacuate h2 + silu."""
                s2 = hp.tile([128, NHB], FP, tag="s2")
                sil2 = hp.tile([64, NHB], BF, tag="sil2")
                for t in range(4):
                    lcols = bass.ts(t, NT)
                    ps2 = pp.tile([128, NT], FP, tag="ps2")
                    for j in range(4):
                        nc.tensor.matmul(ps2[:], w1[j][:], f1[j][:, lcols],
                                         start=(j == 0), stop=False)
                    nc.tensor.matmul(ps2[:], b1[:], sil1[:, lcols],
                                     start=False, stop=True)
                    nc.scalar.activation(s2[:, lcols], ps2[:], AFT.Identity)
                    nc.scalar.activation(sil2[:, lcols], ps2[0:64, :], AFT.Silu)
                return s2, sil2

            def stage_E(b, hb, f2, sil2):
                """L2 matmuls + output staging DMA."""
                for t in range(4):
                    bcols = bass.ts(4*hb + t, NT)
                    lcols = bass.ts(t, NT)
                    ps3 = pp.tile([24, NT], FP, tag="ps3")
                    for j in range(4):
                        nc.tensor.matmul(ps3[:], w2[j][:], f2[j][:, lcols],
                                         start=(j == 0), stop=False)
                    nc.tensor.matmul(ps3[:], b2[:], sil2[:, lcols],
                                     start=False, stop=True)
                    yt = hp.tile([24, NT], FP, tag="yt")
                    nc.scalar.activation(yt[:], ps3[:], AFT.Identity)
                    nc.sync.dma_start(out_d[b, :, bcols], yt[:])

            # Two-deep software pipeline.  The DVE queue is in-order, so the
            # f1 stream runs one block ahead of the f2 stream: DVE order is
            # f1(i), f1(i+1), f2(i), f1(i+2), f2(i+1)... — when f1(i) ends,
            # f1(i+1) is already input-ready, and by the time it ends the L1
            # matmuls + h2 evacuation of block i are done so f2(i) starts
            # without a stall.
            blocks = [(b, hb) for b in range(B_PER_CORE) for hb in range(2)]
            xr_cur = x_load(0)
            xr_nxt = x_load(1) if B_PER_CORE > 1 else None
            f0t_cur = f0_build(xr_cur)
            f0t_nxt = None
            s1_0, sil1_0 = stage_A(f0t_cur, 0)
            pend = [(blocks[0], feats(s1_0, "f1"), sil1_0)]
            for idx in range(len(blocks)):
                if idx + 1 < len(blocks):
                    nb, nhb = blocks[idx + 1]
                    if nhb == 0:
                        f0t_cur = f0t_nxt
                    s1n, sil1n = stage_A(f0t_cur, nhb)
                    f1n = feats(s1n, "f1")
                    pend.append((blocks[idx + 1], f1n, sil1n))
                # build the next batch's f0 right after f1 so a not-yet-ready
                # xr never blocks f1 in the in-order DVE queue; x DMAs were
                # issued a full batch earlier.
                if idx % 2 == 0 and idx // 2 + 1 < B_PER_CORE:
                    f0t_nxt = f0_build(xr_nxt)
                    if idx // 2 + 2 < B_PER_CORE:
                        xr_nxt = x_load(idx // 2 + 2)
                (b, hb), f1, sil1 = pend.pop(0)
                s2, sil2 = stage_C(f1, sil1)
                f2 = feats(s2, "f2")
                stage_E(b, hb, f2, sil2)

    nc.compile()
    return nc


# ------------------------------ entry points -------------------------------

def kernel(x, grid0, coef0, sb0, ss0, grid1, coef1, sb1, ss1, grid2, coef2, sb2, ss2):
    if "nc" not in _CACHE:
        _CACHE["nc"] = _build()
    nc = _CACHE["nc"]

    inputs = {"x": x, "coef0": coef0, "sb0": sb0, "ss0": ss0,
              "coef1": coef1, "sb1": sb1, "ss1": ss1,
              "coef2": coef2, "sb2": sb2, "ss2": ss2}
    consts = _host_weights(inputs)

    xf = np.asarray(x, np.float32).reshape(32, 12, HW)
    maps = []
    for c in range(N_CORES):
        m = dict(consts)
        m["x_in"] = np.ascontiguousarray(xf[c*B_PER_CORE:(c+1)*B_PER_CORE])
        maps.append(m)
    res = run_bass_kernel_spmd(nc, maps, core_ids=list(range(N_CORES)))
    _CACHE["maps"] = maps
    out = np.empty((32, 24, HW), np.float32)
    for c in range(N_CORES):
        out[c*B_PER_CORE:(c+1)*B_PER_CORE] = res.results[c]["y_out"]
    return out.reshape(32, 24, 64, 64)


def _install_ntff_hook():
    import sys, types
    if "antenv.axon_hooks" in sys.modules:
        return
    state = {"hook": None}
    mod = types.ModuleType("antenv.axon_hooks")
    mod.set_axon_ntff_profile_hook = lambda h: state.__setitem__("hook", h)
    mod.get_axon_ntff_profile_hook = lambda: state["hook"]
    sys.modules["antenv.axon_hooks"] = mod
    import antenv
    antenv.axon_hooks = mod
    from trn_agent_boot.trn_boot import _ntff_profile_via_ctypes
    hook = _ntff_profile_via_ctypes("/opt/axon/libaxon_pjrt.so")
    if hook is not None:
        mod.set_axon_ntff_profile_hook(hook)


def profile():
    _install_ntff_hook()
    nc = _CACHE["nc"]
    res = run_bass_kernel_spmd(nc, _CACHE["maps"], core_ids=list(range(N_CORES)),
                               trace=True)
    return res.exec_time_ns, getattr(res, "instructions_and_trace", None)
